# revision 6
# baseline (speedup 1.0000x reference)
"""Decoder block Bass/Tile kernel for TRN2, SPMD over 8 cores.

Sharding: core c = (batch b = c//4, j = c%4). Each core:
  - computes LN1 + K,V for ALL T_kv tokens of its batch (redundant x4, zero comm)
  - handles 512 queries: chunk A = rows [256j, 256j+256), chunk B = rows
    [256(7-j), 256(7-j)+256)  (causal load balance)
  - attention klen padded to a uniform size (1024 for A, 2048 for B) with
    host-provided -60000 masks so the program is identical on all cores
  - proj + residual + LN2 + MLP + residual for its 512 rows
Host gathers the 8 [512, 1024] shards into the full output.

Layouts: "fm" = [feature(partition), token(free)], "rm" = [token, feature].
LN in rm (bn_stats), matmul inputs fm via fp16 DMA-transpose. Matmuls fp16
with fp32 PSUM accumulation. Residual stream fp32.
"""

from contextlib import ExitStack
from dataclasses import dataclass

import numpy as np

import concourse.bass as bass
import concourse.tile as tile
from concourse import mybir
from concourse._compat import with_exitstack

F32 = mybir.dt.float32
F16 = mybir.dt.float16
MASK_NEG = -60000.0


@dataclass
class Cfg:
    D: int = 1024
    DFF: int = 4096
    H: int = 16  # heads
    DH: int = 64  # head dim
    T_kv: int = 2048
    T_q: int = 512  # 2 chunks of CH
    CH: int = 256
    klenA_pad: int = 1024
    klenB_pad: int = 2048
    mmdt: str = "float16"

    @property
    def HP(self):  # head pairs
        return self.H // 2

    @property
    def VA(self):  # augmented V width (dv + ones column per head)
        return self.H * (self.DH + 1)

    @property
    def NKTA(self):
        return self.klenA_pad // 128

    @property
    def NKTB(self):
        return self.klenB_pad // 128

    @property
    def NKT(self):
        return self.NKTA + self.NKTB


def _bcast_ap(ap, p=128):
    """[N] dram AP -> [p, N] with partition stride 0."""
    return bass.AP(tensor=ap.tensor, offset=ap.offset, ap=[[0, p]] + list(ap.ap))


@with_exitstack
def decoder_kernel(ctx: ExitStack, tc: tile.TileContext, cfg: Cfg, io: dict):
    nc = tc.nc
    MD = getattr(mybir.dt, cfg.mmdt)
    D, DFF, H, DH = cfg.D, cfg.DFF, cfg.H, cfg.DH
    HP, VA, CH = cfg.HP, cfg.VA, cfg.CH
    T_kv, T_q = cfg.T_kv, cfg.T_q
    ND = D // 128  # feature tiles
    NFF = DFF // 128
    NTKV = T_kv // 128
    NTQ = T_q // 128
    W2 = 2 * CH  # paired-head free width (512)

    # V chunk width for psum (<=512); VA = H*65
    n_vch = (VA + 511) // 512
    while VA % n_vch != 0:
        n_vch += 1
    VCH = VA // n_vch
    assert VCH <= 512

    const = ctx.enter_context(tc.tile_pool(name="const", bufs=1))
    eps_t = const.tile([128, 1], F32)
    nc.vector.memset(eps_t, 1e-5)
    ones_t = const.tile([1, 64], F32)
    nc.vector.memset(ones_t, 1.0)
    bq_sb = const.tile([128, ND], F32)
    nc.gpsimd.dma_start(out=bq_sb, in_=io["bq"].rearrange("(t p) -> p t", p=128))
    bk_sb = const.tile([128, ND], F32)
    nc.gpsimd.dma_start(out=bk_sb, in_=io["bk"].rearrange("(t p) -> p t", p=128))
    bfc1_sb = const.tile([128, NFF], F32)
    nc.gpsimd.dma_start(out=bfc1_sb, in_=io["bfc1"].rearrange("(t p) -> p t", p=128))
    vb_sb = const.tile([128, VA], F32)
    nc.gpsimd.dma_start(out=vb_sb, in_=_bcast_ap(io["vb"]))

    # ---------------- persistent activations ----------------
    acts = ctx.enter_context(tc.tile_pool(name="acts", bufs=1))
    K_sb = [acts.tile([128, T_kv], MD, tag=f"K{d}", name=f"K{d}") for d in range(ND)]
    Q_sb = [acts.tile([128, 2 * T_q], MD, tag=f"Q{d}", name=f"Q{d}") for d in range(ND)]
    for d in range(ND):
        nc.vector.memset(Q_sb[d], 0.0)
    V_sb = [acts.tile([128, VA], MD, tag=f"V{t}", name=f"V{t}") for t in range(NTKV)]
    O_sb = [acts.tile([128, T_q], MD, tag=f"O{h}", name=f"O{h}") for h in range(HP)]

    # ---------------- LN + transpose helper ----------------
    def ln_transpose(src_dram, src_sb, n_tiles, fm_tiles, pool, stats, tagp):
        for rt in range(n_tiles):
            if src_dram is not None:
                x_t = pool.tile([128, D], F32, tag=f"{tagp}_in")
                nc.gpsimd.dma_start(
                    out=x_t, in_=src_dram[rt * 128 : (rt + 1) * 128, :]
                )
            else:
                x_t = src_sb[rt]
            nsub = D // 512
            st = stats.tile([128, nsub, 6], F32, tag="ln_st")
            for s in range(nsub):
                nc.vector.bn_stats(
                    out=st[:, s, :], in_=x_t[:, s * 512 : (s + 1) * 512]
                )
            mv = stats.tile([128, 2], F32, tag="ln_mv")
            nc.vector.bn_aggr(out=mv, in_=st)
            sd = stats.tile([128, 1], F32, tag="ln_sd")
            nc.scalar.activation(
                out=sd, in_=mv[:, 1:2],
                func=mybir.ActivationFunctionType.Sqrt, bias=eps_t,
            )
            rec = stats.tile([128, 1], F32, tag="ln_rec")
            nc.vector.reciprocal(out=rec, in_=sd)
            xh = pool.tile([128, D], MD, tag=f"{tagp}_xh")
            nc.vector.tensor_scalar(
                out=xh, in0=x_t, scalar1=mv[:, 0:1], scalar2=rec,
                op0=mybir.AluOpType.subtract, op1=mybir.AluOpType.mult,
            )
            for d in range(ND):
                dst = fm_tiles[d][rt // 4][:, (rt % 4) * 128 : (rt % 4 + 1) * 128]
                if mybir.dt.size(MD) == 2:
                    eng = nc.sync if (rt * ND + d) % 2 == 0 else nc.scalar
                    eng.dma_start_transpose(
                        out=dst, in_=xh[:, d * 128 : (d + 1) * 128]
                    )
                else:  # fp32 debug path: strided-AP transpose via plain DMA
                    nc.sync.dma_start(
                        out=dst,
                        in_=xh[:, d * 128 : (d + 1) * 128].rearrange("a b -> b a"),
                    )

    with tc.tile_pool(name="fm", bufs=1) as fmp:
        NCHKV, NCHQ = T_kv // 512, T_q // 512
        xkv_fm = [
            [fmp.tile([128, 512], MD, tag=f"xkvfm{d}_{c}", name=f"xkvfm{d}_{c}")
             for c in range(NCHKV)]
            for d in range(ND)
        ]
        xq_fm = [
            [fmp.tile([128, 512], MD, tag=f"xqfm{d}_{c}", name=f"xqfm{d}_{c}")
             for c in range(NCHQ)]
            for d in range(ND)
        ]
        # ---------------- QKV projections ----------------
        with tc.tile_pool(name="wqk", bufs=3) as wqk, tc.tile_pool(
            name="wv", bufs=1
        ) as wvp, tc.tile_pool(name="psqkv", bufs=4, space="PSUM") as psq:
            # V weights load FIRST (head of the gpsimd DMA queue, no deps)
            wv_sb = [wvp.tile([128, VA], MD, tag=f"wv{kt}", name=f"wv{kt}") for kt in range(ND)]
            for kt in range(ND):
                nc.gpsimd.dma_start(
                    out=wv_sb[kt], in_=io["wv"][kt * 128 : (kt + 1) * 128, :]
                )
            with tc.tile_pool(name="ln1", bufs=3) as lnp, tc.tile_pool(
                name="ln1st", bufs=4
            ) as lnst:
                ln_transpose(io["x_kv"], None, NTKV, xkv_fm, lnp, lnst, "kv")
                ln_transpose(io["x_q"], None, NTQ, xq_fm, lnp, lnst, "q")

            # V (Option 1, consumes fm tiles in LN emission order)
            for tt in range(NTKV):
                for ch in range(n_vch):
                    ps = psq.tile([128, VCH], F32, tag="psv")
                    for kt in range(ND):
                        nc.tensor.matmul(
                            ps,
                            xkv_fm[kt][tt // 4][:, (tt % 4) * 128 : (tt % 4 + 1) * 128],
                            wv_sb[kt][:, ch * VCH : (ch + 1) * VCH],
                            start=(kt == 0),
                            stop=(kt == ND - 1),
                        )
                    nc.vector.tensor_add(
                        out=V_sb[tt][:, ch * VCH : (ch + 1) * VCH],
                        in0=ps,
                        in1=vb_sb[:, ch * VCH : (ch + 1) * VCH],
                    )
            # K then Q (Option 2: weights stationary, fm out)
            for which, wname, bias_sb, fm_src, out_sb, T in (
                ("k", "wk", bk_sb, xkv_fm, K_sb, T_kv),
                ("q", "wq", bq_sb, xq_fm, Q_sb, T_q),
            ):
                for do in range(ND):
                    wb = wqk.tile([128, ND, 128], MD, tag="wqk")
                    nc.gpsimd.dma_start(out=wb, in_=io[wname][do])
                    for tch in range(T // 512):
                        ps = psq.tile([128, 512], F32, tag="psqk")
                        for kt in range(ND):
                            nc.tensor.matmul(
                                ps,
                                wb[:, kt, :],
                                fm_src[kt][tch],
                                start=(kt == 0),
                                stop=(kt == ND - 1),
                            )
                        if which == "k":
                            nc.scalar.activation(
                                out=out_sb[do][:, tch * 512 : (tch + 1) * 512],
                                in_=ps,
                                func=mybir.ActivationFunctionType.Identity,
                                bias=bias_sb[:, do : do + 1],
                            )
                        else:
                            # Q: scatter into per-(chunk, head) blocks with the
                            # complementary head's partitions left zero
                            for ci in range(2):
                                for h in range(2):
                                    blk = (2 * ci + h) * CH
                                    nc.scalar.activation(
                                        out=out_sb[do][
                                            h * 64 : (h + 1) * 64,
                                            blk : blk + CH,
                                        ],
                                        in_=ps[
                                            h * 64 : (h + 1) * 64,
                                            ci * CH : (ci + 1) * CH,
                                        ],
                                        func=mybir.ActivationFunctionType.Identity,
                                        bias=bias_sb[h * 64 : (h + 1) * 64, do : do + 1],
                                    )

    # ---------------- attention + proj ----------------
    mid = ctx.enter_context(tc.tile_pool(name="mid", bufs=1))
    x2_sb = [mid.tile([128, D], F32, tag=f"x2_{t}", name=f"x2_{t}") for t in range(NTQ)]
    xq2_fm = [
        [mid.tile([128, 512], MD, tag=f"xq2fm{d}_{c}", name=f"xq2fm{d}_{c}")
         for c in range(T_q // 512)]
        for d in range(ND)
    ]
    rscr = nc.dram_tensor("rscratch", [2 * HP * 2, CH], F32).ap()
    chunks = [(0, cfg.NKTA, 0), (1, cfg.NKTB, cfg.NKTA)]  # (ci, nkt, mask_off)
    with tc.tile_pool(name="attn_w", bufs=1) as awp:
        # prefetch wproj while attention runs
        wproj_sb = [awp.tile([128, D], MD, tag=f"wp{d}", name=f"wp{d}") for d in range(ND)]
        for d in range(ND):
            nc.gpsimd.dma_start(
                out=wproj_sb[d], in_=io["wproj"][d * 128 : (d + 1) * 128, :]
            )
        with tc.tile_pool(name="attn_m", bufs=1) as mp, tc.tile_pool(
            name="attn_p", bufs=4
        ) as pp, tc.tile_pool(name="attn_ps", bufs=4, space="PSUM"
        ) as aps, tc.tile_pool(name="attn_po", bufs=4, space="PSUM"
        ) as ops:
            for ci, nkt, moff in chunks:
                cc = slice(ci * CH, (ci + 1) * CH)
                masks = []
                for k in range(nkt):
                    m = mp.tile([128, W2], MD, tag=f"mask{ci}_{k}")
                    nc.gpsimd.dma_start(out=m, in_=io["masks"][moff + k, :, :])
                    masks.append(m)
                for hp in range(HP):
                    po = [ops.tile([128, CH], F32, tag="po", name="po") for _ in range(2)]
                    for kti in range(nkt):
                        ps = aps.tile([128, W2], F32, tag="ps_s")
                        kcol = slice(kti * 128, (kti + 1) * 128)
                        for h in range(2):
                            blk = (2 * ci + h) * CH
                            nc.tensor.matmul(
                                ps[:, h * CH : (h + 1) * CH],
                                K_sb[hp][:, kcol],
                                Q_sb[hp][:, blk : blk + CH],
                                start=True, stop=True,
                            )
                        if not (ci == 1 and (kti + 1) * 128 <= cfg.klenB_pad // 2):
                            nc.vector.tensor_add(
                                out=ps, in0=ps, in1=masks[kti]
                            )
                        pt = pp.tile([128, W2], MD, tag="pt")
                        nc.scalar.activation(
                            out=pt, in_=ps,
                            func=mybir.ActivationFunctionType.Exp,
                        )
                        for h in range(2):
                            hg = 2 * hp + h
                            nc.tensor.matmul(
                                po[h][0:65, :],
                                V_sb[kti][:, hg * 65 : hg * 65 + 65],
                                pt[:, h * CH : (h + 1) * CH],
                                start=(kti == 0),
                                stop=(kti == nkt - 1),
                            )
                    # normalize + evict
                    for h in range(2):
                        slot = (ci * HP + hp) * 2 + h
                        r = pp.tile([1, CH], F32, tag="recip")
                        nc.vector.reciprocal(out=r, in_=po[h][64:65, :])
                        nc.sync.dma_start(
                            out=rscr[slot : slot + 1, :], in_=r
                        )
                        # evict numerator scaled by 1/4096 (fits fp16);
                        # normalization happens in one batch at attention end
                        nc.scalar.activation(
                            out=O_sb[hp][h * 64 : (h + 1) * 64, cc],
                            in_=po[h][0:64, :],
                            func=mybir.ActivationFunctionType.Copy,
                            scale=1.0 / 4096.0,
                        )
            # batch normalization of O: bc = 4096/den broadcast via DRAM
            for ci, _, _ in chunks:
                cc = slice(ci * CH, (ci + 1) * CH)
                for hp in range(HP):
                    bc_sb = pp.tile([128, CH], F32, tag="bcsb")
                    for h in range(2):
                        slot = (ci * HP + hp) * 2 + h
                        nc.sync.dma_start(
                            out=bc_sb[h * 64 : (h + 1) * 64, :],
                            in_=bass.AP(
                                tensor=rscr.tensor,
                                offset=rscr.offset + slot * CH,
                                ap=[[0, 64], [1, CH]],
                            ),
                        )
                    nc.vector.tensor_mul(
                        out=O_sb[hp][:, cc], in0=O_sb[hp][:, cc], in1=bc_sb
                    )

        # ---------------- proj + residual ----------------
        with tc.tile_pool(name="proj", bufs=3) as prp, tc.tile_pool(
            name="projps", bufs=4, space="PSUM"
        ) as prps:
            for qt in range(NTQ):
                x_t = prp.tile([128, D], F32, tag="xq_res")
                nc.gpsimd.dma_start(
                    out=x_t, in_=io["x_q"][qt * 128 : (qt + 1) * 128, :]
                )
                for ch2 in range(D // 512):
                    ps = prps.tile([128, 512], F32, tag="pspr")
                    for hp in range(ND):
                        nc.tensor.matmul(
                            ps,
                            O_sb[hp][:, qt * 128 : (qt + 1) * 128],
                            wproj_sb[hp][:, ch2 * 512 : (ch2 + 1) * 512],
                            start=(hp == 0),
                            stop=(hp == ND - 1),
                        )
                    nc.vector.tensor_add(
                        out=x2_sb[qt][:, ch2 * 512 : (ch2 + 1) * 512],
                        in0=ps,
                        in1=x_t[:, ch2 * 512 : (ch2 + 1) * 512],
                    )

    # ---------------- LN2 + transpose ----------------
    with tc.tile_pool(name="ln2", bufs=3) as ln2p, tc.tile_pool(
        name="ln2st", bufs=4
    ) as ln2st:
        ln_transpose(None, x2_sb, NTQ, xq2_fm, ln2p, ln2st, "l2")

    # ---------------- fc1 + gelu + fc2 (pipelined) ----------------
    ghp = ctx.enter_context(tc.tile_pool(name="gh", bufs=1))
    gh_sb = [ghp.tile([128, T_q], MD, tag=f"gh{f}", name=f"gh{f}") for f in range(NFF)]
    with tc.tile_pool(name="fc1w", bufs=3) as f1w, tc.tile_pool(
        name="fc2w", bufs=3
    ) as f2w, tc.tile_pool(name="fc2out", bufs=3) as f2o, tc.tile_pool(
        name="fcps", bufs=3, space="PSUM"
    ) as fps, tc.tile_pool(name="fc2acc", bufs=1, space="PSUM") as f2ps:
        wb2_tiles = {}
        for sweep in range(2):
            accs = {}
            for qt in range(NTQ):
                accs[qt] = f2ps.tile(
                    [128, 512], F32, tag=f"acc{qt}", name=f"acc{qt}"
                )
            for ff in range(NFF):
                if sweep == 0:
                    wb = f1w.tile([128, ND, 128], MD, tag="wfc1")
                    nc.gpsimd.dma_start(out=wb, in_=io["wfc1"][ff])
                    ps = fps.tile([128, T_q], F32, tag="psf1")
                    for kt in range(ND):
                        nc.tensor.matmul(
                            ps, wb[:, kt, :], xq2_fm[kt][0],
                            start=(kt == 0), stop=(kt == ND - 1),
                        )
                    nc.scalar.activation(
                        out=gh_sb[ff], in_=ps,
                        func=mybir.ActivationFunctionType.Gelu,
                        bias=bfc1_sb[:, ff : ff + 1],
                    )
                wb2 = f2w.tile([128, 512], MD, tag="wfc2")
                nc.gpsimd.dma_start(
                    out=wb2,
                    in_=io["wfc2"][
                        ff * 128 : (ff + 1) * 128, sweep * 512 : (sweep + 1) * 512
                    ],
                )
                for qt in range(NTQ):
                    nc.tensor.matmul(
                        accs[qt],
                        gh_sb[ff][:, qt * 128 : (qt + 1) * 128],
                        wb2,
                        start=(ff == 0),
                        stop=(ff == NFF - 1),
                    )
            for qt in range(NTQ):
                o = f2o.tile([128, 512], F32, tag="osb")
                nc.vector.tensor_add(
                    out=o,
                    in0=accs[qt],
                    in1=x2_sb[qt][:, sweep * 512 : (sweep + 1) * 512],
                )
                nc.sync.dma_start(
                    out=io["out"][
                        qt * 128 : (qt + 1) * 128,
                        sweep * 512 : (sweep + 1) * 512,
                    ],
                    in_=o,
                )


# ======================= public entry point =======================

LAST_RESULTS = {}
_CACHE = {}


def kernel(x, ln1_g, ln1_b, w_qkv, w_proj, ln2_g, ln2_b, w_fc1, w_fc2,
           _trace=False):
    """Full-input decoder block on 8 TRN2 NeuronCores; returns full output."""
    from concourse.bass_utils import run_bass_kernel_spmd

    cfg = Cfg()
    in_maps, assemble = host_prep(
        cfg, x, ln1_g, ln1_b, w_qkv, w_proj, ln2_g, ln2_b, w_fc1, w_fc2
    )
    if "nc" not in _CACHE:
        _CACHE["nc"] = build(cfg)
    res = run_bass_kernel_spmd(
        _CACHE["nc"], in_maps, core_ids=list(range(8)), trace=_trace
    )
    LAST_RESULTS["res"] = res
    return assemble(res.results)
